# revision 30
# baseline (speedup 1.0000x reference)
"""MinLSTM fused kernel for Trainium2 (8 NeuronCores, SPMD).

Math: the reference applies cumlogsumexp over the sequence but only the LAST
timestep feeds the output head, so the scan collapses to a single logsumexp
reduction over sequence:

    log_h_last = log_f[S-1] + log(0.5 + sum_s exp(diff_s + log_g(h_s)))
    out = exp(log_h_last) @ w_out.T + b_out

with diff = softplus(-f) - softplus(-i) and per-token term

    exp(diff + log_g(h)) = (1 + e^{-f}) * sigmoid(i) * g(h)
                         = 1/4 * (1+e^{-f}) * (1+tanh(i/2)) * (1+max(2h, tanh(h/2)))

which needs only {exp, tanh} — both in the ACT `exp_and_others` table set
(single table load). The device computes, per core, the partial sum over its
4096 tokens of that product for each of the 1024 hidden channels, fused with
the z = x @ w_in.T matmul (fp8 DoubleRow, fp32 PSUM accumulation). The host
combines partials, applies the exact last-token correction in fp64, and runs
the tiny [4,1024]x[1024,1024] output head.

Sharding: data-parallel over flattened (batch, seq) tokens — core c takes
tokens [c*4096, (c+1)*4096), i.e. batch c//2, sequence half c%2. The sum over
seq is order-independent, so partials combine by addition on host.

Schedule notes (from HW traces): the PE stream is the bottleneck at
~225 ns per 512-token DoubleRow matmul (floor 213), with LDWEIGHTS fully
hidden by the PE reorder window. So the kernel optimizes the edges:
 - dummy matmuls (no deps) ramp the PE p-state during the startup DMA wait
 - per-iteration gate order h,i,f lets the Tanh+DVE chain overlap the f-gate
   matmuls, leaving only Exp + one DVE op after the final matmul
 - startup waits only for the first x block and the first j-stripe's (i,h)
   weight columns
"""

from contextlib import ExitStack

import ml_dtypes
import numpy as np

B, S, D, H = 4, 8192, 1024, 1024
N_CORES = 8
TOK = B * S // N_CORES  # 4096 tokens per core
TB = 512                # token block (matmul moving free dim / PSUM bank)
# half-size first block (shorter startup-critical x DMA) and last block
# (shorter ACT/DVE drain after the final matmul)
BLOCKS = [256] + [TB] * 7 + [256]
NB = len(BLOCKS)
OFFS = [sum(BLOCKS[:k]) for k in range(NB)]
KC = D // 128           # 8 contraction chunks of 128
JC = H // 128           # 8 hidden-channel chunks per gate

WSCALE = 64.0           # w pre-scale so fp8 w values sit in the normal range

_CACHE = {}


def _build_nc():
    import concourse.bacc as bacc
    import concourse.mybir as mybir
    import concourse.tile as tile

    dt = mybir.dt
    AF = mybir.ActivationFunctionType
    ALU = mybir.AluOpType

    in_dt = dt.float8e4
    inv = 1.0 / WSCALE

    nc = bacc.Bacc("TRN2", target_bir_lowering=False)
    xT = nc.dram_tensor("xt", (D, TOK), in_dt, kind="ExternalInput")
    # wtp[j, p, kc, g*128+c] = w_in[g*H + j*128 + c, kc*128 + p]: every
    # j-stripe host-packed in exact SBUF partition-major layout so each
    # stripe DMA is one dense 3KB burst per partition
    wT = nc.dram_tensor("wt", (JC, 128, KC, 384), in_dt, kind="ExternalInput")
    # [partition, j] layout: h-channel = j*128 + p. Contiguous per partition
    # so the final DMA is not a 4-byte-scatter.
    out_sums = nc.dram_tensor("sums", (128, JC, NB), dt.float32, kind="ExternalOutput")

    with tile.TileContext(nc) as tc, ExitStack() as ctx:
        wpool = ctx.enter_context(tc.tile_pool(name="w", bufs=1))
        xpool = ctx.enter_context(tc.tile_pool(name="x", bufs=3))
        gpool = ctx.enter_context(tc.tile_pool(name="g", bufs=3))
        spool = ctx.enter_context(tc.tile_pool(name="s", bufs=1))
        psum = ctx.enter_context(tc.tile_pool(name="psum", bufs=2, space="PSUM"))

        slab = spool.tile([128, JC, NB], dt.float32)

        xT_r = xT[:].rearrange("(kc p) s -> p kc s", p=128)

        # w_all free layout: [kc, j*384 + g*128 + c]
        w_all = wpool.tile([128, KC, JC * 384], in_dt)
        # warmup operand: a DVE memset is the fastest way to materialize it
        # (engines enter "main" at ~6.1us; DMA rings only arm at ~8.3us), so
        # the warmup matmuls can start ~2us before any DMA data could land
        warm = wpool.tile([128, 2, 128], in_dt)
        nc.vector.memset(warm[:], 0.0)

        # Startup-critical transfers: the first x block leads the sync ring
        # while stripe j0's (h,i) columns lead the scalar ring concurrently
        # (h,i before f to match the h,i,f group order below); stripes
        # j1..j7 alternate over the two rings behind them in need order
        # (stripe j is needed ~2.6us per j-iteration in; x block tb only
        # ~21us per tb-iteration in, so all stripes go before x1..x7).
        x_first = xpool.tile([128, KC, BLOCKS[0]], in_dt, tag="x0")
        nc.sync.dma_start(x_first[:, 0:4, :], xT_r[:, 0:4, 0 : BLOCKS[0]])
        nc.scalar.dma_start(x_first[:, 4:8, :], xT_r[:, 4:8, 0 : BLOCKS[0]])
        nc.scalar.dma_start(w_all[:, :, 128:384], wT[0, :, :, 128:384])
        nc.sync.dma_start(w_all[:, :, 0:128], wT[0, :, :, 0:128])
        wq = [nc.sync, nc.scalar]
        for j in range(1, JC):
            wq[j % 2].dma_start(w_all[:, :, j * 384 : (j + 1) * 384], wT[j])

        for tb, (toff, tsz) in enumerate(zip(OFFS, BLOCKS)):
            if tb == 0:
                x_sb = x_first
            else:
                x_sb = xpool.tile([128, KC, tsz], in_dt, tag=f"x{min(tb, 1)}")
                nc.sync.dma_start(x_sb[:], xT_r[:, :, toff : toff + tsz])
            for j in range(JC):
                # i and h share a 2-bank tile so one FD=1024 tanh ACTIVATE
                # covers both; f in its own bank. 3*2 + 2 = 8 PSUM banks.
                ps12 = psum.tile([128, 2, tsz], dt.float32, tag="ps12", bufs=3)
                ps0 = psum.tile([128, tsz], dt.float32, tag="ps0", bufs=2)

                if tb == 0 and j == 0:
                    # dependency-light 128-token matmuls ramp the PE p-state
                    # (needs ~3us of continuous execution) and bridge the gap
                    # until the first x block lands (12.3-14.1us across runs,
                    # trace-measured) — a gap resets the ramp and costs ~3us
                    # of slow matmuls, and the fine 128-token grain wastes at
                    # most ~0.15us when the data arrives early
                    NW = 40
                    for wk in range(NW):
                        nc.tensor.matmul(
                            ps0[:, 0:128],
                            warm[:],
                            warm[:],
                            start=(wk == 0),
                            stop=(wk == NW - 1),
                            perf_mode=mybir.MatmulPerfMode.DoubleRow,
                        )

                def mm_group(out_ap, hs):
                    for kb in range(KC // 2):
                        nc.tensor.matmul(
                            out_ap,
                            w_all[:, 2 * kb : 2 * kb + 2, hs : hs + 128],
                            x_sb[:, 2 * kb : 2 * kb + 2, :],
                            start=(kb == 0),
                            stop=(kb == KC // 2 - 1),
                            perf_mode=mybir.MatmulPerfMode.DoubleRow,
                        )

                # gate order h, i, f: the (i,h) tanh + DVE max/mult chain
                # overlaps the f-gate matmuls; after the last matmul only
                # Exp(f) and the final accumulate op remain. The tanh covers
                # i and h in ONE 1024-wide ACT op — two 512-wide ops would
                # tip ACT past the PE iteration time (ScalarE per-instruction
                # bubble, measured) and stall the PE on PSUM recycling.
                # Exception: the very last iteration splits it so the final
                # serial chain (tanh -> m1 -> w2 -> p -> t) largely runs
                # under the i/f matmuls instead of after them.
                last = tb == NB - 1 and j == JC - 1
                tith = gpool.tile([128, 2, tsz], dt.bfloat16, tag="tith")
                m1 = gpool.tile([128, tsz], dt.bfloat16, tag="m1")
                w2 = gpool.tile([128, tsz], dt.bfloat16, tag="w2")
                p = gpool.tile([128, tsz], dt.bfloat16, tag="p")

                def do_m1w2():
                    nc.vector.scalar_tensor_tensor(
                        m1[:], ps12[:, 1, :], 2.0 * inv, tith[:, 1, :],
                        op0=ALU.mult, op1=ALU.max,
                    )
                    nc.vector.tensor_scalar_add(w2[:], m1[:], 1.0)

                def do_p():
                    nc.vector.scalar_tensor_tensor(
                        p[:], tith[:, 0, :], 1.0, w2[:], op0=ALU.add, op1=ALU.mult
                    )

                mm_group(ps12[:, 1, :], j * 384 + 256)  # h
                if last:
                    nc.scalar.activation(
                        tith[:, 1, :], ps12[:, 1, :], AF.Tanh, scale=0.5 * inv
                    )
                    do_m1w2()
                mm_group(ps12[:, 0, :], j * 384 + 128)  # i
                if last:
                    nc.scalar.activation(
                        tith[:, 0, :], ps12[:, 0, :], AF.Tanh, scale=0.5 * inv
                    )
                    do_p()
                else:
                    nc.scalar.activation(tith[:], ps12[:], AF.Tanh, scale=0.5 * inv)
                    do_m1w2()
                    do_p()
                mm_group(ps0[:], j * 384)  # f
                a = gpool.tile([128, tsz], dt.bfloat16, tag="a")
                nc.scalar.activation(a[:], ps0[:], AF.Exp, scale=-inv)
                # t = (1+e^{-f}) * p, accumulated over the 512 tokens
                t = gpool.tile([128, tsz], dt.bfloat16, tag="t")
                nc.vector.scalar_tensor_tensor(
                    t[:],
                    a[:],
                    1.0,
                    p[:],
                    op0=ALU.add,
                    op1=ALU.mult,
                    accum_out=slab[:, j, tb : tb + 1],
                )

        # ship the per-block partials raw (host sums the 8 blocks per
        # channel) — skips an on-device reduce on the critical tail path
        nc.scalar.dma_start(out_sums[:], slab[:])

    nc.compile()
    return nc


def _get_nc():
    if "nc" not in _CACHE:
        _CACHE["nc"] = _build_nc()
    return _CACHE["nc"]


def _softplus(v):
    return np.log1p(np.exp(-np.abs(v))) + np.maximum(v, 0.0)


def kernel(x, w_in, w_out, b_out, _return_results=False, _trace=False):
    from concourse.bass_utils import run_bass_kernel_spmd

    x = np.asarray(x)
    w_in = np.asarray(w_in)
    w_out = np.asarray(w_out)
    b_out = np.asarray(b_out)

    cast_dt = ml_dtypes.float8_e4m3  # TRN FP8_EXP4: max ±240, inf above

    def cast(a):
        return np.clip(a, -240.0, 240.0).astype(cast_dt)

    # wtp[j, p, kc, g*128+c] = (w_in*WSCALE)[g*H + j*128 + c, kc*128 + p]
    # (partition-major SBUF layout so each stripe DMA is a dense burst)
    wq = (w_in * WSCALE).reshape(3, JC, 128, KC, 128)  # [g, j, c, kc, p]
    wT = cast(np.ascontiguousarray(wq.transpose(1, 4, 3, 0, 2).reshape(JC, 128, KC, 384)))
    xf = x.reshape(B * S, D)
    in_maps = []
    for c in range(N_CORES):
        xs = xf[c * TOK : (c + 1) * TOK]  # [TOK, D]
        xt = cast(np.ascontiguousarray(xs.T))  # [D, TOK]
        in_maps.append({"xt": xt, "wt": wT})

    nc = _get_nc()
    # the first execution of a freshly compiled NEFF occasionally hits a
    # transient NRT exec error on this setup — retry once
    try:
        res = run_bass_kernel_spmd(
            nc, in_maps, core_ids=list(range(N_CORES)), trace=_trace
        )
    except Exception:
        import time as _time

        _time.sleep(2.0)
        res = run_bass_kernel_spmd(
            nc, in_maps, core_ids=list(range(N_CORES)), trace=False
        )

    # sums[p, j] -> channel h = j*128 + p
    parts = [
        np.asarray(r["sums"]).astype(np.float64).sum(axis=-1).T.reshape(H)
        for r in res.results
    ]
    Ssum = np.stack([parts[2 * b] + parts[2 * b + 1] for b in range(B)]) * 0.25

    # exact last-token factor in fp64 (host): log_f[S-1] = -softplus(diff[S-1])
    z_last = x[:, -1, :].astype(np.float64) @ w_in.astype(np.float64).T
    f_l, i_l = z_last[:, :H], z_last[:, H : 2 * H]
    diff_l = _softplus(-f_l) - _softplus(-i_l)
    h_last = np.exp(-_softplus(diff_l) + np.log(0.5 + Ssum))
    out = (h_last @ w_out.astype(np.float64).T + b_out.astype(np.float64)).astype(
        np.float32
    )
    if _return_results:
        return out, res
    return out


# revision 31
# speedup vs baseline: 1.0203x; 1.0203x over previous
"""MinLSTM fused kernel for Trainium2 (8 NeuronCores, SPMD).

Math: the reference applies cumlogsumexp over the sequence but only the LAST
timestep feeds the output head, so the scan collapses to a single logsumexp
reduction over sequence:

    log_h_last = log_f[S-1] + log(0.5 + sum_s exp(diff_s + log_g(h_s)))
    out = exp(log_h_last) @ w_out.T + b_out

with diff = softplus(-f) - softplus(-i) and per-token term

    exp(diff + log_g(h)) = (1 + e^{-f}) * sigmoid(i) * g(h)
                         = 1/4 * (1+e^{-f}) * (1+tanh(i/2)) * (1+max(2h, tanh(h/2)))

which needs only {exp, tanh} — both in the ACT `exp_and_others` table set
(single table load). The device computes, per core, the partial sum over its
4096 tokens of that product for each of the 1024 hidden channels, fused with
the z = x @ w_in.T matmul (fp8 DoubleRow, fp32 PSUM accumulation). The host
combines partials, applies the exact last-token correction in fp64, and runs
the tiny [4,1024]x[1024,1024] output head.

Sharding: data-parallel over flattened (batch, seq) tokens — core c takes
tokens [c*4096, (c+1)*4096), i.e. batch c//2, sequence half c%2. The sum over
seq is order-independent, so partials combine by addition on host.

Schedule notes (from HW traces): the PE stream is the bottleneck at
~225 ns per 512-token DoubleRow matmul (floor 213), with LDWEIGHTS fully
hidden by the PE reorder window. So the kernel optimizes the edges:
 - dummy matmuls (no deps) ramp the PE p-state during the startup DMA wait
 - per-iteration gate order h,i,f lets the Tanh+DVE chain overlap the f-gate
   matmuls, leaving only Exp + one DVE op after the final matmul
 - startup waits only for the first x block and the first j-stripe's (i,h)
   weight columns
"""

from contextlib import ExitStack

import ml_dtypes
import numpy as np

B, S, D, H = 4, 8192, 1024, 1024
N_CORES = 8
TOK = B * S // N_CORES  # 4096 tokens per core
TB = 512                # token block (matmul moving free dim / PSUM bank)
BLOCKS = [TB] * 8
NB = len(BLOCKS)
OFFS = [sum(BLOCKS[:k]) for k in range(NB)]
KC = D // 128           # 8 contraction chunks of 128
JC = H // 128           # 8 hidden-channel chunks per gate

WSCALE = 64.0           # w pre-scale so fp8 w values sit in the normal range

_CACHE = {}


def _build_nc():
    import concourse.bacc as bacc
    import concourse.mybir as mybir
    import concourse.tile as tile

    dt = mybir.dt
    AF = mybir.ActivationFunctionType
    ALU = mybir.AluOpType

    in_dt = dt.float8e4
    inv = 1.0 / WSCALE

    nc = bacc.Bacc("TRN2", target_bir_lowering=False)
    xT = nc.dram_tensor("xt", (D, TOK), in_dt, kind="ExternalInput")
    # wtp[j, p, kc, g*128+c] = w_in[g*H + j*128 + c, kc*128 + p]: every
    # j-stripe host-packed in exact SBUF partition-major layout so each
    # stripe DMA is one dense 3KB burst per partition
    wT = nc.dram_tensor("wt", (JC, 128, KC, 384), in_dt, kind="ExternalInput")
    # [partition, j] layout: h-channel = j*128 + p. Contiguous per partition
    # so the final DMA is not a 4-byte-scatter.
    out_sums = nc.dram_tensor("sums", (128, JC, NB), dt.float32, kind="ExternalOutput")

    with tile.TileContext(nc) as tc, ExitStack() as ctx:
        wpool = ctx.enter_context(tc.tile_pool(name="w", bufs=1))
        xpool = ctx.enter_context(tc.tile_pool(name="x", bufs=3))
        gpool = ctx.enter_context(tc.tile_pool(name="g", bufs=3))
        spool = ctx.enter_context(tc.tile_pool(name="s", bufs=1))
        psum = ctx.enter_context(tc.tile_pool(name="psum", bufs=2, space="PSUM"))

        slab = spool.tile([128, JC, NB], dt.float32)

        xT_r = xT[:].rearrange("(kc p) s -> p kc s", p=128)

        # w_all free layout: [kc, j*384 + g*128 + c]
        w_all = wpool.tile([128, KC, JC * 384], in_dt)
        # warmup operand: a DVE memset is the fastest way to materialize it
        # (engines enter "main" at ~6.1us; DMA rings only arm at ~8.3us), so
        # the warmup matmuls can start ~2us before any DMA data could land
        warm = wpool.tile([128, 2, 128], in_dt)
        nc.vector.memset(warm[:], 0.0)

        # Startup-critical transfers: the first x block leads the sync ring
        # while stripe j0's (h,i) columns lead the scalar ring concurrently
        # (h,i before f to match the h,i,f group order below); stripes
        # j1..j7 alternate over the two rings behind them in need order
        # (stripe j is needed ~2.6us per j-iteration in; x block tb only
        # ~21us per tb-iteration in, so all stripes go before x1..x7).
        x_first = xpool.tile([128, KC, BLOCKS[0]], in_dt, tag="x0")
        nc.sync.dma_start(x_first[:, 0:4, :], xT_r[:, 0:4, 0 : BLOCKS[0]])
        nc.scalar.dma_start(x_first[:, 4:8, :], xT_r[:, 4:8, 0 : BLOCKS[0]])
        nc.scalar.dma_start(w_all[:, :, 128:384], wT[0, :, :, 128:384])
        nc.sync.dma_start(w_all[:, :, 0:128], wT[0, :, :, 0:128])
        wq = [nc.sync, nc.scalar]
        for j in range(1, JC):
            wq[j % 2].dma_start(w_all[:, :, j * 384 : (j + 1) * 384], wT[j])

        for tb, (toff, tsz) in enumerate(zip(OFFS, BLOCKS)):
            if tb == 0:
                x_sb = x_first
            else:
                x_sb = xpool.tile([128, KC, tsz], in_dt, tag=f"x{min(tb, 1)}")
                nc.sync.dma_start(x_sb[:], xT_r[:, :, toff : toff + tsz])
            for j in range(JC):
                # i and h share a 2-bank tile so one FD=1024 tanh ACTIVATE
                # covers both; f in its own bank. 3*2 + 2 = 8 PSUM banks.
                ps12 = psum.tile([128, 2, tsz], dt.float32, tag="ps12", bufs=3)
                ps0 = psum.tile([128, tsz], dt.float32, tag="ps0", bufs=2)

                if tb == 0 and j == 0:
                    # dependency-light 128-token matmuls ramp the PE p-state
                    # (needs ~3us of continuous execution) and bridge the gap
                    # until the first x block lands (12.3-14.1us across runs,
                    # trace-measured) — a gap resets the ramp and costs ~3us
                    # of slow matmuls, and the fine 128-token grain wastes at
                    # most ~0.15us when the data arrives early
                    NW = 46
                    for wk in range(NW):
                        nc.tensor.matmul(
                            ps0[:, 0:128],
                            warm[:],
                            warm[:],
                            start=(wk == 0),
                            stop=(wk == NW - 1),
                            perf_mode=mybir.MatmulPerfMode.DoubleRow,
                        )

                def mm_group(out_ap, hs):
                    for kb in range(KC // 2):
                        nc.tensor.matmul(
                            out_ap,
                            w_all[:, 2 * kb : 2 * kb + 2, hs : hs + 128],
                            x_sb[:, 2 * kb : 2 * kb + 2, :],
                            start=(kb == 0),
                            stop=(kb == KC // 2 - 1),
                            perf_mode=mybir.MatmulPerfMode.DoubleRow,
                        )

                # gate order h, i, f: the (i,h) tanh + DVE max/mult chain
                # overlaps the f-gate matmuls; after the last matmul only
                # Exp(f) and the final accumulate op remain. The tanh covers
                # i and h in ONE 1024-wide ACT op — two 512-wide ops would
                # tip ACT past the PE iteration time (ScalarE per-instruction
                # bubble, measured) and stall the PE on PSUM recycling.
                # Exception: the very last iteration splits it so the final
                # serial chain (tanh -> m1 -> w2 -> p -> t) largely runs
                # under the i/f matmuls instead of after them.
                last = tb == NB - 1 and j == JC - 1
                tith = gpool.tile([128, 2, tsz], dt.bfloat16, tag="tith")
                m1 = gpool.tile([128, tsz], dt.bfloat16, tag="m1")
                w2 = gpool.tile([128, tsz], dt.bfloat16, tag="w2")
                p = gpool.tile([128, tsz], dt.bfloat16, tag="p")

                def do_m1w2():
                    nc.vector.scalar_tensor_tensor(
                        m1[:], ps12[:, 1, :], 2.0 * inv, tith[:, 1, :],
                        op0=ALU.mult, op1=ALU.max,
                    )
                    nc.vector.tensor_scalar_add(w2[:], m1[:], 1.0)

                def do_p():
                    nc.vector.scalar_tensor_tensor(
                        p[:], tith[:, 0, :], 1.0, w2[:], op0=ALU.add, op1=ALU.mult
                    )

                mm_group(ps12[:, 1, :], j * 384 + 256)  # h
                if last:
                    nc.scalar.activation(
                        tith[:, 1, :], ps12[:, 1, :], AF.Tanh, scale=0.5 * inv
                    )
                    do_m1w2()
                mm_group(ps12[:, 0, :], j * 384 + 128)  # i
                if last:
                    nc.scalar.activation(
                        tith[:, 0, :], ps12[:, 0, :], AF.Tanh, scale=0.5 * inv
                    )
                    do_p()
                else:
                    nc.scalar.activation(tith[:], ps12[:], AF.Tanh, scale=0.5 * inv)
                    do_m1w2()
                    do_p()
                mm_group(ps0[:], j * 384)  # f
                a = gpool.tile([128, tsz], dt.bfloat16, tag="a")
                nc.scalar.activation(a[:], ps0[:], AF.Exp, scale=-inv)
                # t = (1+e^{-f}) * p, accumulated over the 512 tokens
                t = gpool.tile([128, tsz], dt.bfloat16, tag="t")
                nc.vector.scalar_tensor_tensor(
                    t[:],
                    a[:],
                    1.0,
                    p[:],
                    op0=ALU.add,
                    op1=ALU.mult,
                    accum_out=slab[:, j, tb : tb + 1],
                )

        # ship the per-block partials raw (host sums the 8 blocks per
        # channel) — skips an on-device reduce on the critical tail path
        nc.scalar.dma_start(out_sums[:], slab[:])

    nc.compile()
    return nc


def _get_nc():
    if "nc" not in _CACHE:
        _CACHE["nc"] = _build_nc()
    return _CACHE["nc"]


def _softplus(v):
    return np.log1p(np.exp(-np.abs(v))) + np.maximum(v, 0.0)


def kernel(x, w_in, w_out, b_out, _return_results=False, _trace=False):
    from concourse.bass_utils import run_bass_kernel_spmd

    x = np.asarray(x)
    w_in = np.asarray(w_in)
    w_out = np.asarray(w_out)
    b_out = np.asarray(b_out)

    cast_dt = ml_dtypes.float8_e4m3  # TRN FP8_EXP4: max ±240, inf above

    def cast(a):
        return np.clip(a, -240.0, 240.0).astype(cast_dt)

    # wtp[j, p, kc, g*128+c] = (w_in*WSCALE)[g*H + j*128 + c, kc*128 + p]
    # (partition-major SBUF layout so each stripe DMA is a dense burst)
    wq = (w_in * WSCALE).reshape(3, JC, 128, KC, 128)  # [g, j, c, kc, p]
    wT = cast(np.ascontiguousarray(wq.transpose(1, 4, 3, 0, 2).reshape(JC, 128, KC, 384)))
    xf = x.reshape(B * S, D)
    in_maps = []
    for c in range(N_CORES):
        xs = xf[c * TOK : (c + 1) * TOK]  # [TOK, D]
        xt = cast(np.ascontiguousarray(xs.T))  # [D, TOK]
        in_maps.append({"xt": xt, "wt": wT})

    nc = _get_nc()
    # the first execution of a freshly compiled NEFF occasionally hits a
    # transient NRT exec error on this setup — retry once
    try:
        res = run_bass_kernel_spmd(
            nc, in_maps, core_ids=list(range(N_CORES)), trace=_trace
        )
    except Exception:
        import time as _time

        _time.sleep(2.0)
        res = run_bass_kernel_spmd(
            nc, in_maps, core_ids=list(range(N_CORES)), trace=False
        )

    # sums[p, j] -> channel h = j*128 + p
    parts = [
        np.asarray(r["sums"]).astype(np.float64).sum(axis=-1).T.reshape(H)
        for r in res.results
    ]
    Ssum = np.stack([parts[2 * b] + parts[2 * b + 1] for b in range(B)]) * 0.25

    # exact last-token factor in fp64 (host): log_f[S-1] = -softplus(diff[S-1])
    z_last = x[:, -1, :].astype(np.float64) @ w_in.astype(np.float64).T
    f_l, i_l = z_last[:, :H], z_last[:, H : 2 * H]
    diff_l = _softplus(-f_l) - _softplus(-i_l)
    h_last = np.exp(-_softplus(diff_l) + np.log(0.5 + Ssum))
    out = (h_last @ w_out.astype(np.float64).T + b_out.astype(np.float64)).astype(
        np.float32
    )
    if _return_results:
        return out, res
    return out
